# revision 14
# baseline (speedup 1.0000x reference)
"""Trainium2 Bass kernel for nn_MnistDetector (nms_detection).

Sharding: core k handles image k//2 and ROI half k%2 (128 ROIs/core).
Backbone+head replicated per image pair; regions computed as two batched
separable-bilinear matmul stages; IoU on-device with anchors on partitions.
"""
import dataclasses
import sys

sys.path.insert(0, '/opt/trn_rl_repo')
sys.path.insert(0, '/opt/pypackages')

import numpy as np
import concourse.bass as bass
import concourse.bacc as bacc
import concourse.mybir as mybir
from concourse import tile
from concourse.bass_utils import run_bass_kernel_spmd

F32 = mybir.dt.float32
I32 = mybir.dt.int32
U8 = mybir.dt.uint8
AF = mybir.ActivationFunctionType
ALU = mybir.AluOpType
AX = mybir.AxisListType

HP = WP = 22
RX = RY = 28
NROI = 128          # rois per core
GRP = 8             # rois per stage-1 group
NG = NROI // GRP    # 16 groups
NCHI = 32           # c_hi chunks (128 channels / 4)

TRACE = False
LAST_RESULTS = None


def _dview(ap, offset, dims):
    return dataclasses.replace(ap, offset=offset, ap=[list(d) for d in dims])


def _load_weight(nc, pool, w_d, K, M, name):
    """(K, M) weight -> SBUF tile [128, nch, M]; chunk c rows = min(128, K-128c)."""
    nch = (K + 127) // 128
    t = pool.tile([128, nch, M], F32, tag=f"w_{name}", name=f"w_{name}")
    for c in range(nch):
        rows = min(128, K - 128 * c)
        nc.sync.dma_start(t[0:rows, c, :], w_d[128 * c:128 * c + rows, :])
    return t


def _im2col_sbuf(nc, pool, tag, src_ap, C, Hs, Ws, Ho, Wo, tileN,
                 r0=0, r1=None):
    """im2col chunk tiles from SBUF src AP with logical shape (C, Hs, Ws),
    covering output rows [r0, r1) of the Ho x Wo conv output.

    Returns [(tile, rows)]. Rows ordered (tap, ci), tap = kh*3+kw.
    """
    if r1 is None:
        r1 = Ho
    nr = r1 - r0
    N = nr * Wo
    K = 9 * C
    nch = (K + 127) // 128
    tiles = []
    base_off = src_ap.offset
    for c in range(nch):
        rows = min(128, K - 128 * c)
        tl = pool.tile([128, tileN], F32, tag=tag, name=f"{tag}_{c}")
        t0 = (128 * c) // C
        t1 = (128 * c + rows) // C
        for t in range(t0, t1):
            kh, kw = t // 3, t % 3
            off = base_off + (kh + r0) * Ws + kw
            src_dims = [[Hs * Ws, C], [Ws, nr], [1, Wo]]
            d0 = (t - t0) * C
            nc.sync.dma_start(tl[d0:d0 + C, 0:N],
                              _dview(src_ap, off, src_dims))
        tiles.append((tl, rows))
    return tiles


def _conv(nc, psp, wt, bt, tiles, M, Wo, row_chunks, act_out, act=AF.Relu):
    """K-chunk accumulating matmuls + ACT eviction, by output-row chunks.

    act_out(r0, r1) -> dest AP shaped (M, r1-r0, Wo).
    """
    for (q0, q1) in row_chunks:
        pm = psp.tile([M, q1 - q0, Wo], F32, tag="cpsum", name="cpsum")
        for c, (tl, rows) in enumerate(tiles):
            nc.tensor.matmul(pm[:], wt[0:rows, c, 0:M],
                             tl[0:rows, q0 * Wo:q1 * Wo],
                             start=(c == 0), stop=(c == len(tiles) - 1))
        nc.scalar.activation(act_out(q0, q1), pm[:], act, bias=bt)


def _rowchunks(H, step):
    return [(i, min(i + step, H)) for i in range(0, H, step)]


def build_nc():
    nc = bacc.Bacc("TRN2", target_bir_lowering=False, debug=False)

    # ---------------- dram I/O ----------------
    x_d = nc.dram_tensor("x_img", (112, 112), F32, kind="ExternalInput")
    wspec = [(9, 16), (144, 16), (144, 32), (288, 32), (288, 64), (576, 64),
             (576, 128), (1152, 128)]
    w_d = [nc.dram_tensor(f"w{i}T", s, F32, kind="ExternalInput")
           for i, s in enumerate(wspec)]
    b_d = [nc.dram_tensor(f"b{i}", (s[1], 1), F32, kind="ExternalInput")
           for i, s in enumerate(wspec)]
    r0T_d = nc.dram_tensor("r0T", (1152, 256), F32, kind="ExternalInput")
    rb0_d = nc.dram_tensor("rb0", (256, 1), F32, kind="ExternalInput")
    r1T_d = nc.dram_tensor("r1T", (256, 45), F32, kind="ExternalInput")
    rb1_d = nc.dram_tensor("rb1", (45, 1), F32, kind="ExternalInput")
    wx_d = nc.dram_tensor("wx", (22, NROI * RY), F32, kind="ExternalInput")
    wyb_d = nc.dram_tensor("wyb", (NROI, 88, 112), F32, kind="ExternalInput")
    anc_d = nc.dram_tensor("anc", (128, 6, 35), F32, kind="ExternalInput")
    gt_d = nc.dram_tensor("gt", (128, 6, 4), F32, kind="ExternalInput")
    id_d = nc.dram_tensor("ident", (128, 128), F32, kind="ExternalInput")

    feats_o = nc.dram_tensor("feats_o", (128, 484), F32, kind="ExternalOutput")
    conf_o = nc.dram_tensor("conf_o", (9, 484), F32, kind="ExternalOutput")
    diffs_o = nc.dram_tensor("diffs_o", (36, 484), F32, kind="ExternalOutput")
    reg_o = nc.dram_tensor("reg_o", (64, 2, 4, RX, NCHI, RY), F32,
                           kind="ExternalOutput")
    iomax_o = nc.dram_tensor("iomax_o", (128, 35), F32, kind="ExternalOutput")
    ioarg_o = nc.dram_tensor("ioarg_o", (128, 35), I32, kind="ExternalOutput")

    p1_d = nc.dram_tensor("p1buf", (16, 112, 112), F32)  # internal padded

    with tile.TileContext(nc) as tc:
        with tc.tile_pool(name="pp", bufs=1) as pp:
            ident = pp.tile([128, 128], F32)
            nc.sync.dma_start(ident[:], id_d[:])
            wx_s = pp.tile([22, NROI * RY], F32)
            nc.sync.dma_start(wx_s[:], wx_d[:])
            p8 = pp.tile([128, 24, 24], F32)

            # =============== backbone + head + iou (scoped pools) ===========
            with (
                tc.tile_pool(name="wp", bufs=1) as wp,
                tc.tile_pool(name="bbp", bufs=1) as bbp,
                tc.tile_pool(name="wk", bufs=1) as wk,
                tc.tile_pool(name="strip", bufs=2) as stp,
                tc.tile_pool(name="ic01", bufs=3) as ic01,
                tc.tile_pool(name="ic23", bufs=3) as ic23,
                tc.tile_pool(name="ic45", bufs=6) as ic45,
                tc.tile_pool(name="ic678", bufs=10) as ic678,
                tc.tile_pool(name="bps", bufs=2, space="PSUM") as bps,
            ):
                wts = [_load_weight(nc, wp, w_d[i], *wspec[i], name=str(i))
                       for i in range(8)]
                bts = []
                for i, s in enumerate(wspec):
                    bt = wp.tile([s[1], 1], F32, tag=f"b_{i}", name=f"b_{i}")
                    nc.sync.dma_start(bt[:], b_d[i][:])
                    bts.append(bt)
                r0T = _load_weight(nc, wp, r0T_d, 1152, 256, name="r0")
                rb0 = wp.tile([128, 2, 1], F32, tag="rb0")
                nc.sync.dma_start(
                    rb0[:], rb0_d[:].rearrange("(c p) o -> p c o", p=128))
                r1T = _load_weight(nc, wp, r1T_d, 256, 45, name="r1")
                rb1 = wp.tile([45, 1], F32, tag="rb1")
                nc.sync.dma_start(rb1[:], rb1_d[:])

                # ---------------- IOU ----------------
                anc_t = bbp.tile([128, 6, 35], F32)
                gt_t = bbp.tile([128, 6, 4], F32)
                nc.sync.dma_start(anc_t[:], anc_d[:])
                nc.sync.dma_start(gt_t[:], gt_d[:])

                def ab(f):
                    return anc_t[:, f, :].rearrange(
                        "p (a o) -> p a o", o=1).broadcast_to((128, 35, 4))

                def gb(f):
                    return gt_t[:, f, :].rearrange(
                        "p (o g) -> p o g", o=1).broadcast_to((128, 35, 4))

                SH = [128, 35, 4]
                ix1 = wk.tile(SH, F32, tag="i1")
                iy1 = wk.tile(SH, F32, tag="i2")
                ix2 = wk.tile(SH, F32, tag="i3")
                iy2 = wk.tile(SH, F32, tag="i4")
                nc.vector.tensor_tensor(ix1[:], ab(0), gb(0), op=ALU.max)
                nc.vector.tensor_tensor(iy1[:], ab(1), gb(1), op=ALU.max)
                nc.vector.tensor_tensor(ix2[:], ab(2), gb(2), op=ALU.min)
                nc.vector.tensor_tensor(iy2[:], ab(3), gb(3), op=ALU.min)
                nc.vector.tensor_tensor(ix2[:], ix2[:], ix1[:],
                                        op=ALU.subtract)
                nc.vector.tensor_tensor(iy2[:], iy2[:], iy1[:],
                                        op=ALU.subtract)
                nc.vector.tensor_scalar_max(ix2[:], ix2[:], 0.0)
                nc.vector.tensor_scalar_max(iy2[:], iy2[:], 0.0)
                inter = wk.tile(SH, F32, tag="i5")
                nc.vector.tensor_tensor(inter[:], ix2[:], iy2[:], op=ALU.mult)
                den = wk.tile(SH, F32, tag="i6")
                nc.vector.tensor_tensor(den[:], gb(4), ab(4), op=ALU.add)
                nc.vector.tensor_tensor(den[:], den[:], inter[:],
                                        op=ALU.subtract)
                rec = wk.tile(SH, F32, tag="i7")
                nc.vector.reciprocal(rec[:], den[:])
                iou = bbp.tile(SH, F32)
                nc.vector.tensor_tensor(iou[:], inter[:], rec[:], op=ALU.mult)

                def cross_partition(red_in, op, tagp):
                    m4 = wk.tile([128, 4], F32, tag=tagp + "a",
                                 name=tagp + "a")
                    nc.vector.tensor_reduce(
                        m4[:], red_in.rearrange("p a g -> p g a"),
                        axis=AX.X, op=op)
                    pt1 = bps.tile([4, 128], F32, tag="iops", name="iops1")
                    nc.tensor.transpose(pt1[:], m4[:], ident[:, :])
                    mg = wk.tile([4, 1], F32, tag=tagp + "b", name=tagp + "b")
                    nc.vector.tensor_reduce(mg[:], pt1[:], axis=AX.X, op=op)
                    mgr = wk.tile([4, 128], F32, tag=tagp + "c",
                                  name=tagp + "c")
                    nc.vector.tensor_copy(mgr[:], mg[:].broadcast_to((4, 128)))
                    pt2 = bps.tile([128, 4], F32, tag="iops", name="iops2")
                    nc.tensor.transpose(pt2[:], mgr[:], ident[0:4, 0:4])
                    mf = wk.tile([128, 4], F32, tag=tagp + "d",
                                 name=tagp + "d")
                    nc.vector.tensor_copy(mf[:], pt2[:])
                    return mf

                big = bbp.tile([128, 35, 4], F32)
                nc.vector.memset(big[:], 1e9)

                mf = cross_partition(iou[:], ALU.max, "cm")
                mfb = mf[:].rearrange("p (o g) -> p o g",
                                      o=1).broadcast_to(SH)
                msk = wk.tile(SH, U8, tag="msk")
                nc.vector.tensor_tensor(msk[:], iou[:], mfb, op=ALU.is_ge)
                sel = wk.tile(SH, F32, tag="sel")
                nc.vector.select(sel[:], msk[:], ab(5), big[:])
                bf = cross_partition(sel[:], ALU.min, "cb")
                bfb = bf[:].rearrange("p (o g) -> p o g",
                                      o=1).broadcast_to(SH)
                eqb = wk.tile(SH, U8, tag="msk2")
                nc.vector.tensor_tensor(eqb[:], ab(5), bfb, op=ALU.is_equal)
                rt = wk.tile(SH, F32, tag="rt")
                nc.vector.tensor_scalar_mul(rt[:], eqb[:], 0.6)
                nc.vector.tensor_tensor(iou[:], iou[:], rt[:], op=ALU.max)
                iomax = bbp.tile([128, 35], F32)
                nc.vector.tensor_reduce(iomax[:], iou[:], axis=AX.X,
                                        op=ALU.max)
                nc.sync.dma_start(iomax_o[:], iomax[:])
                mb2 = iomax[:].rearrange("p (a o) -> p a o",
                                         o=1).broadcast_to(SH)
                ge2 = wk.tile(SH, U8, tag="msk")
                nc.vector.tensor_tensor(ge2[:], iou[:], mb2, op=ALU.is_ge)
                selg = wk.tile(SH, F32, tag="sel")
                nc.vector.select(selg[:], ge2[:], gb(5), big[:])
                argf = wk.tile([128, 35], F32, tag="argf")
                nc.vector.tensor_reduce(argf[:], selg[:], axis=AX.X,
                                        op=ALU.min)
                argi = wk.tile([128, 35], I32, tag="argi")
                nc.vector.tensor_copy(argi[:], argf[:])
                nc.sync.dma_start(ioarg_o[:], argi[:])

                # ---------------- conv0 ----------------
                zrow = wk.tile([16, 112], F32, tag="zrow")
                nc.vector.memset(zrow[:], 0.0)
                nc.sync.dma_start(
                    _dview(p1_d[:], 0,
                           [[12544, 16], [111 * 112, 2], [1, 112]]),
                    zrow[:].rearrange("c (o w) -> c o w", o=1).broadcast_to(
                        (16, 2, 112)))
                strips = _rowchunks(110, 14)
                for (r0, r1) in strips:
                    nr = r1 - r0
                    ic0 = ic01.tile([9, 14 * 110], F32, tag="ic01",
                                    name="ic0")
                    for kh in range(3):
                        nc.sync.dma_start(
                            ic0[kh * 3:kh * 3 + 3, 0:nr * 110],
                            _dview(x_d[:], (kh + r0) * 112,
                                   [[1, 3], [112, nr], [1, 110]]))
                    s0 = stp.tile([16, 14, 112], F32, tag="s0")
                    nc.vector.memset(s0[:, :, 0:112:111], 0.0)
                    for (q0, q1) in _rowchunks(nr, 4):
                        pm = bps.tile([16, 4, 110], F32, tag="cpsum",
                                      name="c0psum")
                        nc.tensor.matmul(
                            pm[:, 0:q1 - q0, :], wts[0][0:9, 0, :],
                            ic0[0:9, q0 * 110:q1 * 110],
                            start=True, stop=True)
                        nc.scalar.activation(
                            s0[:, q0:q1, 1:111], pm[:, 0:q1 - q0, :],
                            AF.Relu, bias=bts[0][:])
                    nc.sync.dma_start(
                        _dview(p1_d[:], (1 + r0) * 112,
                               [[12544, 16], [112, nr], [1, 112]]),
                        s0[:, 0:nr, :])

                # ---------------- conv1 + maxpool1 ----------------
                pool1 = bbp.tile([16, 55, 55], F32)
                for (r0, r1) in strips:
                    nr = r1 - r0
                    ics = []
                    for c, rows in [(0, 128), (1, 16)]:
                        tl = ic01.tile([128, 14 * 110], F32, tag="ic01",
                                       name=f"ic1_{c}")
                        t0, t1 = (128 * c) // 16, (128 * c + rows) // 16
                        for t in range(t0, t1):
                            kh, kw = t // 3, t % 3
                            nc.sync.dma_start(
                                tl[(t - t0) * 16:(t - t0 + 1) * 16,
                                   0:nr * 110],
                                _dview(p1_d[:], (kh + r0) * 112 + kw,
                                       [[12544, 16], [112, nr], [1, 110]]))
                        ics.append((tl, rows))
                    c1s = stp.tile([16, 14, 110], F32, tag="c1s")
                    for (q0, q1) in _rowchunks(nr, 4):
                        pm = bps.tile([16, 4, 110], F32, tag="cpsum",
                                      name="c1psum")
                        for c, (tl, rows) in enumerate(ics):
                            nc.tensor.matmul(
                                pm[:, 0:q1 - q0, :], wts[1][0:rows, c, :],
                                tl[0:rows, q0 * 110:q1 * 110],
                                start=(c == 0), stop=(c == 1))
                        nc.scalar.activation(
                            c1s[:, q0:q1, :], pm[:, 0:q1 - q0, :],
                            AF.Relu, bias=bts[1][:])
                    pr = nr // 2
                    po = r0 // 2
                    t1_ = wk.tile([16, 7, 55], F32, tag="mp1")
                    t2_ = wk.tile([16, 7, 55], F32, tag="mp2")
                    nc.vector.tensor_tensor(
                        t1_[:, 0:pr, :], c1s[:, 0:nr:2, 0:110:2],
                        c1s[:, 0:nr:2, 1:110:2], op=ALU.max)
                    nc.vector.tensor_tensor(
                        t2_[:, 0:pr, :], c1s[:, 1:nr:2, 0:110:2],
                        c1s[:, 1:nr:2, 1:110:2], op=ALU.max)
                    nc.vector.tensor_tensor(
                        pool1[:, po:po + pr, :], t1_[:, 0:pr, :],
                        t2_[:, 0:pr, :], op=ALU.max)

                # ---------------- conv2 -> p3 center ----------------
                p3 = bbp.tile([32, 55, 55], F32)
                nc.gpsimd.memset(p3[:], 0.0)
                for (h0, h1) in [(0, 27), (27, 53)]:
                    tiles2 = _im2col_sbuf(nc, ic23, "ic23", pool1[:],
                                          16, 55, 55, 53, 53, 27 * 53, h0, h1)
                    _conv(nc, bps, wts[2], bts[2][:], tiles2, 32, 53,
                          _rowchunks(h1 - h0, 9),
                          lambda a, b, h0=h0: p3[:, 1 + h0 + a:1 + h0 + b,
                                                 1:54])

                # ---------------- conv3 -> c3s ----------------
                c3s = bbp.tile([32, 53, 53], F32)
                for (h0, h1) in [(0, 27), (27, 53)]:
                    tiles3 = _im2col_sbuf(nc, ic23, "ic23", p3[:],
                                          32, 55, 55, 53, 53, 27 * 53, h0, h1)
                    _conv(nc, bps, wts[3], bts[3][:], tiles3, 32, 53,
                          _rowchunks(h1 - h0, 9),
                          lambda a, b, h0=h0: c3s[:, h0 + a:h0 + b, :])

                # ---------------- maxpool2 ----------------
                pool2 = bbp.tile([32, 26, 26], F32)
                mpa = wk.tile([32, 26, 26], F32, tag="mp3")
                mpb = wk.tile([32, 26, 26], F32, tag="mp4")
                nc.vector.tensor_tensor(
                    mpa[:], c3s[:, 0:52:2, 0:52:2], c3s[:, 0:52:2, 1:52:2],
                    op=ALU.max)
                nc.vector.tensor_tensor(
                    mpb[:], c3s[:, 1:52:2, 0:52:2], c3s[:, 1:52:2, 1:52:2],
                    op=ALU.max)
                nc.vector.tensor_tensor(pool2[:], mpa[:], mpb[:], op=ALU.max)

                # ---------------- conv4 -> p5 center ----------------
                p5 = bbp.tile([64, 26, 26], F32)
                nc.gpsimd.memset(p5[:], 0.0)
                tiles4 = _im2col_sbuf(nc, ic45, "ic45", pool2[:],
                                      32, 26, 26, 24, 24, 576)
                _conv(nc, bps, wts[4], bts[4][:], tiles4, 64, 24,
                      _rowchunks(24, 12),
                      lambda a, b: p5[:, 1 + a:1 + b, 1:25])

                # ---------------- conv5 -> c5s ----------------
                c5s = bbp.tile([64, 24, 24], F32)
                tiles5 = _im2col_sbuf(nc, ic45, "ic45", p5[:],
                                      64, 26, 26, 24, 24, 576)
                _conv(nc, bps, wts[5], bts[5][:], tiles5, 64, 24,
                      _rowchunks(24, 12),
                      lambda a, b: c5s[:, a:b, :])

                # ---------------- conv6 -> p7 center ----------------
                p7 = bbp.tile([128, 24, 24], F32)
                nc.gpsimd.memset(p7[:], 0.0)
                tiles6 = _im2col_sbuf(nc, ic678, "ic678", c5s[:],
                                      64, 24, 24, 22, 22, 484)
                _conv(nc, bps, wts[6], bts[6][:], tiles6, 128, 22,
                      [(0, 22)],
                      lambda a, b: p7[:, 1:23, 1:23])

                # ---------------- conv7 -> p8 center (DVE evict) ------------
                nc.vector.memset(p8[:], 0.0)
                tiles7 = _im2col_sbuf(nc, ic678, "ic678", p7[:],
                                      128, 24, 24, 22, 22, 484)
                pm7 = bps.tile([128, 22, 22], F32, tag="cpsum", name="c7psum")
                for c, (tl, rows) in enumerate(tiles7):
                    nc.tensor.matmul(pm7[:], wts[7][0:rows, c, :],
                                     tl[0:rows, :],
                                     start=(c == 0), stop=(c == len(tiles7) - 1))
                nc.vector.tensor_scalar(p8[:, 1:23, 1:23], pm7[:],
                                        bts[7][:], 0.0,
                                        op0=ALU.add, op1=ALU.max)
                nc.sync.dma_start(feats_o[:].rearrange("c (a b) -> c a b",
                                                       a=22),
                                  p8[:, 1:23, 1:23])

                # ---------------- rpn head ----------------
                tiles8 = _im2col_sbuf(nc, ic678, "ic678", p8[:],
                                      128, 24, 24, 22, 22, 484)
                r0s = [wk.tile([128, 484], F32, tag=f"r0s{m}",
                               name=f"r0s{m}") for m in range(2)]
                for m in range(2):
                    pmr = bps.tile([128, 484], F32, tag="cpsum",
                                   name=f"r0psum{m}")
                    for c, (tl, rows) in enumerate(tiles8):
                        nc.tensor.matmul(
                            pmr[:], r0T[0:rows, c, 128 * m:128 * (m + 1)],
                            tl[0:rows, :],
                            start=(c == 0), stop=(c == len(tiles8) - 1))
                    nc.scalar.activation(r0s[m][:], pmr[:], AF.Relu,
                                         bias=rb0[:, m, :])
                bbs = wk.tile([45, 484], F32, tag="bbs")
                pmb = bps.tile([45, 484], F32, tag="cpsum", name="r1psum")
                for m in range(2):
                    nc.tensor.matmul(pmb[:], r1T[:, m, :], r0s[m][:],
                                     start=(m == 0), stop=(m == 1))
                nc.scalar.activation(bbs[:], pmb[:], AF.Identity,
                                     bias=rb1[:])
                conf = wk.tile([9, 484], F32, tag="conf")
                nc.scalar.activation(conf[:], bbs[0:9, :], AF.Sigmoid)
                nc.sync.dma_start(conf_o[:], conf[:])
                nc.sync.dma_start(diffs_o[:], bbs[9:45, :])

            # ================= regions =================
            with (
                tc.tile_pool(name="rp", bufs=1) as rp,
                tc.tile_pool(name="rw", bufs=2) as rw,
                tc.tile_pool(name="up", bufs=3) as up,
                tc.tile_pool(name="ps1", bufs=2, space="PSUM") as ps1,
                tc.tile_pool(name="ps2", bufs=2, space="PSUM") as ps2,
                tc.tile_pool(name="pst", bufs=1, space="PSUM") as pst,
            ):
                # lhsT1: L1[w, chi, cl, h] = feats[chi*4+cl, h, w]
                L1 = rp.tile([22, NCHI, 4, 22], F32)
                sc96 = rw.tile([32, 24, 24], F32, tag="sc96", name="sc96")
                nc.sync.dma_start(sc96[:], p8[96:128, :, :])
                for blk in range(4):
                    if blk < 3:
                        src = p8[32 * blk:32 * blk + 32, :, :]
                        base = 32 * blk
                    else:
                        src = sc96[:]
                        base = 0
                    ptr = pst.tile([22, 22, 32], F32, tag="ptr", name="ptr")
                    for h in range(22):
                        nc.tensor.transpose(
                            ptr[:, h, :],
                            src[:, 1 + h, 1:23],
                            ident[base:base + 32, base:base + 32])
                    nc.vector.tensor_copy(
                        L1[:, 8 * blk:8 * blk + 8, :, :],
                        ptr[:].rearrange("w h (s c) -> w s c h", s=8))

                t_half = [rp.tile([88, 16, GRP, RY], F32, tag=f"th{h}",
                                  name=f"th{h}") for h in range(2)]
                for g in range(NG):
                    wyb_g = rw.tile([88, GRP, 112], F32, tag="wyb",
                                    name="wyb")
                    nc.sync.dma_start(
                        wyb_g[:], wyb_d[g * GRP:(g + 1) * GRP].rearrange(
                            "r p m -> p r m"))
                    for chi in range(NCHI):
                        pm1 = ps1.tile([88, GRP, RY], F32, tag="pm1",
                                       name="pm1")
                        nc.tensor.matmul(
                            pm1[:], L1[:, chi, :, :],
                            wx_s[:, g * GRP * RY:(g + 1) * GRP * RY],
                            start=True, stop=True)
                        if chi < 16:
                            nc.vector.tensor_copy(
                                t_half[0][:, chi, :, :], pm1[:])
                        else:
                            nc.scalar.activation(
                                t_half[1][:, chi - 16, :, :], pm1[:], AF.Copy)
                    for q in range(GRP):
                        roi = g * GRP + q
                        if q % 2 == 0:
                            u = up.tile([112, 2, 2, 16, RY], F32, tag="u",
                                        name="u")
                        for h in range(2):
                            pm2 = ps2.tile([112, 16, RY], F32, tag=f"pm2{h}",
                                           name=f"pm2{h}")
                            nc.tensor.matmul(
                                pm2[:], wyb_g[:, q, :], t_half[h][:, :, q, :],
                                start=True, stop=True)
                            if h == 0:
                                nc.vector.tensor_copy(
                                    u[:, q % 2, 0, :, :], pm2[:])
                            else:
                                nc.scalar.activation(
                                    u[:, q % 2, 1, :, :], pm2[:], AF.Copy)
                        if q % 2 == 1:
                            pair = roi // 2
                            nc.sync.dma_start(
                                reg_o[pair].rearrange(
                                    "r cl i ch j -> (cl i) r (ch j)"),
                                u[:])

    nc.compile()
    return nc


# ====================== host side ======================

_NC = None


def _get_nc():
    global _NC
    if _NC is None:
        _NC = build_nc()
    return _NC


def _coords(lo, hi, n):
    lo = lo.astype(np.float32)
    L = (hi - lo + 1).astype(np.float32)
    g = (np.arange(n, dtype=np.float32) + np.float32(0.5))[None, :]
    g = g * L[:, None] / np.float32(n) - np.float32(0.5)
    g = np.clip(g, np.float32(0.0), (L - np.float32(1.0))[:, None])
    g = g + lo[:, None]
    i0 = np.floor(g).astype(np.int32)
    f = (g - i0.astype(np.float32)).astype(np.float32)
    i0 = np.minimum(i0, hi[:, None].astype(np.int32))
    i1 = np.minimum(i0 + 1, hi[:, None].astype(np.int32))
    return i0, i1, f


def _interp_matrix(lo, hi, n):
    """(R,) int boxes -> (R, n, 22) interpolation matrices."""
    R = lo.shape[0]
    i0, i1, f = _coords(lo, hi, n)
    W = np.zeros((R, n, HP), dtype=np.float32)
    r = (np.arange(R)[:, None] * np.ones((1, n), np.int64)).astype(np.int64)
    j = (np.ones((R, 1), np.int64) * np.arange(n)[None, :]).astype(np.int64)
    np.add.at(W, (r, j, i0), (1.0 - f).astype(np.float32))
    np.add.at(W, (r, j, i1), f.astype(np.float32))
    return W


def _anchors():
    sizes = (0.15, 0.45, 0.75)
    ratios = (0.5, 1.0, 2.0)
    ar = (np.arange(HP, dtype=np.float32) + np.float32(0.5)) / np.float32(HP)
    cy = np.repeat(ar, WP).reshape(HP, WP)
    cx = np.tile(ar, HP).reshape(HP, WP)
    boxes = []
    for s in sizes:
        for r in ratios:
            w = np.float32(s * np.sqrt(r))
            h = np.float32(s / np.sqrt(r))
            boxes.append(np.stack([cx - w / np.float32(2),
                                   cy - h / np.float32(2),
                                   cx + w / np.float32(2),
                                   cy + h / np.float32(2)]).astype(np.float32))
    return np.stack(boxes, 1).reshape(4, -1).astype(np.float32)


def kernel(x, boxes, gt_boxes, fw0, fb0, fw1, fb1, fw2, fb2, fw3, fb3,
           fw4, fb4, fw5, fb5, fw6, fb6, fw7, fb7, rw0, rb0, rw1, rb1):
    global LAST_RESULTS
    nc = _get_nc()
    x = np.asarray(x, dtype=np.float32)
    boxes = np.asarray(boxes, dtype=np.int32)
    gt_boxes = np.asarray(gt_boxes, dtype=np.float32)
    fws = [np.asarray(w, np.float32) for w in
           (fw0, fw1, fw2, fw3, fw4, fw5, fw6, fw7)]
    fbs = [np.asarray(b, np.float32) for b in
           (fb0, fb1, fb2, fb3, fb4, fb5, fb6, fb7)]
    rw0 = np.asarray(rw0, np.float32)
    rb0v = np.asarray(rb0, np.float32)
    rw1 = np.asarray(rw1, np.float32)
    rb1v = np.asarray(rb1, np.float32)

    base = {}
    for i, w in enumerate(fws):
        base[f"w{i}T"] = np.ascontiguousarray(
            w.transpose(2, 3, 1, 0).reshape(-1, w.shape[0]))
        base[f"b{i}"] = fbs[i].reshape(-1, 1)
    base["r0T"] = np.ascontiguousarray(
        rw0.transpose(2, 3, 1, 0).reshape(1152, 256))
    base["rb0"] = rb0v.reshape(-1, 1)
    base["r1T"] = np.ascontiguousarray(
        rw1.transpose(2, 3, 1, 0).reshape(256, 45))
    base["rb1"] = rb1v.reshape(-1, 1)
    base["ident"] = np.eye(128, dtype=np.float32)

    anc = _anchors()
    A = anc.shape[1]
    anc_p = np.zeros((4, 4480), np.float32)
    anc_p[:, :A] = anc
    aarea = (anc_p[2] - anc_p[0]) * (anc_p[3] - anc_p[1])
    aidx = np.arange(4480, dtype=np.float32)
    anc_t = np.stack([anc_p[0], anc_p[1], anc_p[2], anc_p[3], aarea, aidx])
    base["anc"] = np.ascontiguousarray(
        anc_t.reshape(6, 35, 128).transpose(2, 0, 1))

    in_maps = []
    B = x.shape[0]
    for k in range(8):
        img, half = k // 2, k % 2
        m = dict(base)
        m["x_img"] = np.ascontiguousarray(x[img, 0])
        b = boxes[img][:, half * NROI:(half + 1) * NROI]
        Wy = _interp_matrix(b[0], b[2], RX)     # (128, 28, 22)
        Wx = _interp_matrix(b[1], b[3], RY)     # (128, 28, 22)
        m["wx"] = np.ascontiguousarray(
            Wx.transpose(2, 0, 1).reshape(22, NROI * RY))
        wyb = np.zeros((NROI, 4, HP, 4, RX), np.float32)
        WyT = Wy.transpose(0, 2, 1)             # (128, 22, 28)
        for cl in range(4):
            wyb[:, cl, :, cl, :] = WyT
        m["wyb"] = np.ascontiguousarray(wyb.reshape(NROI, 88, 112))
        gt = gt_boxes[img]                      # (4, G=4) rows x1,y1,x2,y2
        garea = ((gt[2] - gt[0]) * (gt[3] - gt[1])).astype(np.float32)
        gtile = np.stack([gt[0], gt[1], gt[2], gt[3],
                          garea + np.float32(1e-9),
                          np.arange(4, dtype=np.float32)])
        m["gt"] = np.ascontiguousarray(
            np.broadcast_to(gtile[None], (128, 6, 4)))
        in_maps.append(m)

    res = run_bass_kernel_spmd(nc, in_maps, core_ids=list(range(8)),
                               trace=TRACE)
    LAST_RESULTS = res

    feats = np.stack([res.results[2 * b]["feats_o"].reshape(128, HP, WP)
                      for b in range(B)])
    conf = np.stack([res.results[2 * b]["conf_o"].reshape(9, HP, WP)
                     for b in range(B)])
    diffs = np.stack([res.results[2 * b]["diffs_o"].reshape(4, 9, HP, WP)
                      for b in range(B)])
    regions = np.empty((B, 256, 128, RX, RY), np.float32)
    for k in range(8):
        img, half = k // 2, k % 2
        st = res.results[k]["reg_o"].reshape(NROI, 4, RX, NCHI, RY)
        regions[img, half * NROI:(half + 1) * NROI] = (
            st.transpose(0, 3, 1, 2, 4).reshape(NROI, 128, RX, RY))
    iou_max = np.stack([
        res.results[2 * b]["iomax_o"].T.reshape(-1)[:A] for b in range(B)])
    iou_argmax = np.stack([
        res.results[2 * b]["ioarg_o"].T.reshape(-1)[:A] for b in range(B)]
    ).astype(np.int32)
    return feats, conf, diffs, regions, iou_max, iou_argmax
